# revision 1
# baseline (speedup 1.0000x reference)
"""Trainium2 Bass kernel for a 2-layer LSTM (H=50) + linear head with
autoregressive future steps. Data-parallel over 8 NeuronCores (batch sharded).

Design "Q" (sigmoid-only, quadrant-packed):
  - Per core 2048 samples: lo block (0:1024) on partitions 0:50, hi block
    (1024:2048) on partitions 64:114; free dim = sample-within-block, two
    512-wide passes per step.
  - Gate pre-acts per cell-pass land in one 4-bank PSUM tile [128, 2048]
    (free = i|f|o|g x 512) -> one sigmoid (i,f,o) + one tanh (g) per
    cell-pass; sigmoid and tanh share one ACT table set.
  - tanh(c1(t)) and tanh(c2(t-1)) share one tanh op via an interleaved c-tile
    (free = c1p0|c2p0|c1p1|c2p1); h2(t-1) materializes early in step t, just
    in time for cell2(t)'s matmuls, whose W2B part also computes
    y(t-1) = Wl h2(t-1) + bl as column 50 of the o-gate chunk (M=51).
  - Future phase (y feeds back as x) uses a dedicated small matmul per step.
"""

import sys
import os
import numpy as np

for _p in ("/opt/trn_rl_repo", "/root/.axon_site/_ro/trn_rl_repo"):
    if os.path.isdir(_p) and _p not in sys.path:
        sys.path.insert(0, _p)
        break

from contextlib import ExitStack

import concourse.bass as bass
import concourse.mybir as mybir
import concourse.tile as tile
from concourse import bacc
from concourse.bass import ds, ts
from concourse.bass_utils import run_bass_kernel_spmd

FP16 = mybir.dt.float16
FP32 = mybir.dt.float32
AF = mybir.ActivationFunctionType
ALU = mybir.AluOpType

H = 50
B = 16384
NCORES = 8
BC = B // NCORES          # 2048 samples per core
HALF = 1024               # samples per partition-block (lo/hi)
F = 512                   # free width per pass (one PSUM bank of fp32)

# our gate chunk order: i, f, o(+y), g ; torch block order: i, f, g, o
GATE_SRC = [0, 1, 3, 2]


def _build_nc(T, FUT):
    TT = T + FUT
    nc = bacc.Bacc("TRN2", target_bir_lowering=False, debug=False,
                   num_devices=NCORES)

    xT = nc.dram_tensor("xT", [T, BC], FP16, kind="ExternalInput")
    W1 = nc.dram_tensor("W1", [128, 200], FP16, kind="ExternalInput")
    W2A = nc.dram_tensor("W2A", [128, 201], FP16, kind="ExternalInput")
    W2B = nc.dram_tensor("W2B", [128, 201], FP16, kind="ExternalInput")
    WLY = nc.dram_tensor("WLY", [128, 1], FP16, kind="ExternalInput")
    ONES = nc.dram_tensor("ONES", [1, HALF], FP16, kind="ExternalInput")
    yT = nc.dram_tensor("yT", [TT, BC], FP16, kind="ExternalOutput")
    DUM = nc.dram_tensor("DUM", [128, 32], FP16, kind="ExternalOutput")
    debug_yb = os.environ.get("DEBUG_YB") == "1"
    if debug_yb:
        DBG = nc.dram_tensor("DBG", [2, 128, F], FP16, kind="ExternalOutput")

    with tile.TileContext(nc) as tc, ExitStack() as ctx:
        const = ctx.enter_context(tc.tile_pool(name="const", bufs=1))
        state = ctx.enter_context(tc.tile_pool(name="state", bufs=1))
        sa_p = ctx.enter_context(tc.tile_pool(name="sa", bufs=4))
        sb_p = ctx.enter_context(tc.tile_pool(name="sb", bufs=6))
        sc_p = ctx.enter_context(tc.tile_pool(name="scp", bufs=6))
        tmp_p = ctx.enter_context(tc.tile_pool(name="tmp", bufs=10))
        y_p = ctx.enter_context(tc.tile_pool(name="yp", bufs=4))
        pg1_p = ctx.enter_context(tc.tile_pool(name="pg1", bufs=1, space="PSUM"))
        pg2_p = ctx.enter_context(tc.tile_pool(name="pg2", bufs=1, space="PSUM"))

        w1 = const.tile([128, 200], FP16, tag="w1")
        w2a = const.tile([128, 201], FP16, tag="w2a")
        w2b = const.tile([128, 201], FP16, tag="w2b")
        wly = const.tile([128, 1], FP16, tag="wly")
        nc.sync.dma_start(out=w1[:], in_=W1.ap())
        nc.sync.dma_start(out=w2a[:], in_=W2A.ap())
        nc.sync.dma_start(out=w2b[:], in_=W2B.ap())
        nc.sync.dma_start(out=wly[:], in_=WLY.ap())

        # S1: [h1(0:50) | x(50) | 1(51)] lo; hi at 64:114, 114, 115
        s1 = [state.tile([128, HALF], FP16, tag=f"s1_{b}", name=f"s1_{b}")
              for b in range(2)]
        # S2: [h2(0:50) | 1(50)] lo; hi at 64:114, 114
        s2 = [state.tile([128, HALF], FP16, tag=f"s2_{b}", name=f"s2_{b}")
              for b in range(2)]
        # C state, free = [c1p0 | c2p0 | c1p1 | c2p1] (512 each)
        dst = state.tile([128, 2048], FP16, tag="dst")

        for b in range(2):
            nc.vector.memset(s1[b][:], 0.0)
            nc.vector.memset(s2[b][:], 0.0)
            nc.sync.dma_start(out=s1[b][51:52, :], in_=ONES.ap())
            nc.sync.dma_start(out=s1[b][115:116, :], in_=ONES.ap())
            nc.sync.dma_start(out=s2[b][50:51, :], in_=ONES.ap())
            nc.sync.dma_start(out=s2[b][114:115, :], in_=ONES.ap())
        nc.vector.memset(dst[:], 0.0)

        def dma_x_in(t):
            sl = s1[t % 2]
            nc.sync.dma_start(out=sl[50:51, :], in_=xT.ap()[t:t + 1, 0:HALF])
            nc.sync.dma_start(out=sl[114:115, :],
                              in_=xT.ap()[t:t + 1, HALF:2 * HALF])

        dma_x_in(0)
        if T > 1:
            dma_x_in(1)

        sb_prev = [None, None]

        for t in range(TT):
            cur, nxt = t % 2, (t + 1) % 2
            S1c, S1n = s1[cur], s1[nxt]
            S2c, S2n = s2[cur], s2[nxt]
            future = t >= T - 1
            sb_new = [None, None]

            for p in range(2):
                fs = ds(p * F, F)
                d1s = ds(p * 1024, F)
                d2s = ds(p * 1024 + F, F)

                # ---- cell1 matmuls: gates1 = W1 @ [h1; x; 1] ----
                pg1 = pg1_p.tile([128, 2048], FP32, tag="pg1", name="pg1")
                for G in range(4):
                    gf = ts(G, F)
                    gc = ts(G, H)
                    nc.tensor.matmul(pg1[0:50, gf], w1[0:52, gc],
                                     S1c[0:52, fs], start=True, stop=True)
                    nc.tensor.matmul(pg1[64:114, gf], w1[64:116, gc],
                                     S1c[64:116, fs], start=True, stop=True)

                sA = sa_p.tile([128, 2048], FP16, tag="sA", name="sA")
                nc.scalar.activation(sA[:, :], pg1[:, :], AF.Sigmoid)

                # ---- cell1 elementwise: d1 = i*(4*sig_g-2) + f*d1 (d=2c) ----
                g2t = tmp_p.tile([128, F], FP16, tag="g2t", name="g2t")
                nc.vector.tensor_scalar(g2t[0:114, :], sA[0:114, 1536:2048],
                                        4.0, 2.0, ALU.mult, ALU.subtract)
                at = tmp_p.tile([128, F], FP16, tag="at", name="at")
                nc.vector.tensor_mul(at[0:114, :], sA[0:114, 0:512],
                                     g2t[0:114, :])
                ft = tmp_p.tile([128, F], FP16, tag="ft", name="ft")
                nc.vector.tensor_mul(ft[0:114, :], sA[0:114, 512:1024],
                                     dst[0:114, d1s])
                nc.vector.tensor_add(dst[0:114, d1s], at[0:114, :],
                                     ft[0:114, :])

                # ---- shared sigmoid over [d1(t,p) | d2(t-1,p)] ----
                scs = sc_p.tile([128, 1024], FP16, tag="scs", name="scs")
                nc.scalar.activation(scs[:, :], dst[:, ds(p * 1024, 1024)],
                                     AF.Sigmoid)

                # h2(t-1) = o2(t-1) * (2*sig(d2(t-1))-1) -> S2c (gates cell2)
                if 0 < t < T:
                    sBp = sb_prev[p]
                    tc2 = tmp_p.tile([128, F], FP16, tag="tc2", name="tc2")
                    nc.vector.tensor_scalar(tc2[0:114, :],
                                            scs[0:114, 512:1024],
                                            2.0, 1.0, ALU.mult, ALU.subtract)
                    nc.vector.tensor_mul(S2c[0:50, fs], sBp[0:50, 1024:1536],
                                         tc2[0:50, :])
                    nc.vector.tensor_mul(S2c[64:114, fs],
                                         sBp[64:114, 1024:1536],
                                         tc2[64:114, :])

                # h1(t) = o1 * (2*sig(d1)-1) -> S1n rows 0:50 / 64:114
                tc1 = tmp_p.tile([128, F], FP16, tag="tc1", name="tc1")
                nc.vector.tensor_scalar(tc1[0:114, :], scs[0:114, 0:512],
                                        2.0, 1.0, ALU.mult, ALU.subtract)
                nc.vector.tensor_mul(S1n[0:50, fs], sA[0:50, 1024:1536],
                                     tc1[0:50, :])
                nc.vector.tensor_mul(S1n[64:114, fs], sA[64:114, 1024:1536],
                                     tc1[64:114, :])

                # ---- cell2 matmuls: gates2 = W2A @ h1 + W2B @ [h2; 1] ----
                pg2 = pg2_p.tile([128, 2048], FP32, tag="pg2", name="pg2")
                for G in range(4):
                    gf = ts(G, F)
                    if G == 2:
                        cab, mb = ds(100, 51), 51
                    else:
                        cab = ds(G * H + (1 if G == 3 else 0), H)
                        mb = 50
                    nc.tensor.matmul(pg2[0:mb, gf], w2a[0:50, cab],
                                     S1n[0:50, fs], start=True, stop=False)
                    nc.tensor.matmul(pg2[0:mb, gf], w2b[0:51, cab],
                                     S2c[0:51, fs], start=False, stop=True)
                    nc.tensor.matmul(pg2[64:64 + mb, gf], w2a[64:114, cab],
                                     S1n[64:114, fs], start=True, stop=False)
                    nc.tensor.matmul(pg2[64:64 + mb, gf], w2b[64:115, cab],
                                     S2c[64:115, fs], start=False, stop=True)

                sB = sb_p.tile([128, 2048], FP16, tag="sB", name="sB")
                nc.scalar.activation(sB[:, :], pg2[:, :], AF.Sigmoid)
                sb_new[p] = sB

                # raw y(t-1) extraction from o-chunk rows 50 / 114
                if 0 < t < T:
                    yb = y_p.tile([128, F], FP16, tag="yb", name="yb")
                    nc.vector.tensor_copy(yb[:, :], pg2[:, 1024:1536])
                    nc.sync.dma_start(out=yT.ap()[t - 1:t, p * F:(p + 1) * F],
                                      in_=yb[50:51, :])
                    nc.sync.dma_start(
                        out=yT.ap()[t - 1:t, HALF + p * F:HALF + (p + 1) * F],
                        in_=yb[114:115, :])
                    if debug_yb and t == 1:
                        nc.sync.dma_start(out=DBG.ap()[p], in_=yb[:, :])

                # ---- cell2 elementwise: d2 = i2*(4*sig_g2-2) + f2*d2 ----
                g2t2 = tmp_p.tile([128, F], FP16, tag="g2t2", name="g2t2")
                nc.vector.tensor_scalar(g2t2[0:114, :], sB[0:114, 1536:2048],
                                        4.0, 2.0, ALU.mult, ALU.subtract)
                at2 = tmp_p.tile([128, F], FP16, tag="at2", name="at2")
                nc.vector.tensor_mul(at2[0:114, :], sB[0:114, 0:512],
                                     g2t2[0:114, :])
                ft2 = tmp_p.tile([128, F], FP16, tag="ft2", name="ft2")
                nc.vector.tensor_mul(ft2[0:114, :], sB[0:114, 512:1024],
                                     dst[0:114, d2s])
                nc.vector.tensor_add(dst[0:114, d2s], at2[0:114, :],
                                     ft2[0:114, :])

                if future:
                    # h2(t) and y(t) inline (y feeds x(t+1))
                    scf = sc_p.tile([128, F], FP16, tag="scf", name="scf")
                    nc.scalar.activation(scf[:, :], dst[:, d2s], AF.Sigmoid)
                    tcf = tmp_p.tile([128, F], FP16, tag="tcf", name="tcf")
                    nc.vector.tensor_scalar(tcf[0:114, :], scf[0:114, :],
                                            2.0, 1.0, ALU.mult, ALU.subtract)
                    nc.vector.tensor_mul(S2n[0:50, fs], sB[0:50, 1024:1536],
                                         tcf[0:50, :])
                    nc.vector.tensor_mul(S2n[64:114, fs],
                                         sB[64:114, 1024:1536],
                                         tcf[64:114, :])
                    pgy = pg1_p.tile([128, F], FP32, tag="pg1", name="pgy")
                    nc.tensor.matmul(pgy[0:1, :], wly[0:51, :], S2n[0:51, fs],
                                     start=True, stop=True)
                    nc.tensor.matmul(pgy[64:65, :], wly[64:115, :],
                                     S2n[64:115, fs], start=True, stop=True)
                    ycf = y_p.tile([128, F], FP16, tag="ycf", name="ycf")
                    nc.vector.tensor_copy(ycf[:, :], pgy[:, :])
                    nc.sync.dma_start(out=yT.ap()[t:t + 1, p * F:(p + 1) * F],
                                      in_=ycf[0:1, :])
                    nc.sync.dma_start(
                        out=yT.ap()[t:t + 1, HALF + p * F:HALF + (p + 1) * F],
                        in_=ycf[64:65, :])
                    if t + 1 < TT:
                        nc.sync.dma_start(out=S1n[50:51, fs], in_=ycf[0:1, :])
                        nc.sync.dma_start(out=S1n[114:115, fs],
                                          in_=ycf[64:65, :])

            sb_prev = sb_new
            if t + 2 < T:
                dma_x_in(t + 2)

        dum = y_p.tile([128, 32], FP16, tag="dum", name="dum")
        nc.vector.memset(dum[:], 0.0)
        nc.sync.dma_start(out=DUM.ap(), in_=dum[:])

    nc.compile()
    return nc


def _prep_weights(Wih1, Whh1, bih1, bhh1, Wih2, Whh2, bih2, bhh2, Wl, bl):
    b1 = (bih1 + bhh1).astype(np.float32)
    b2 = (bih2 + bhh2).astype(np.float32)

    W1 = np.zeros((128, 200), np.float32)
    W2A = np.zeros((128, 201), np.float32)
    W2B = np.zeros((128, 201), np.float32)
    WLY = np.zeros((128, 1), np.float32)
    for G, src in enumerate(GATE_SRC):
        blk = slice(src * H, (src + 1) * H)
        gm = 2.0 if G == 3 else 1.0
        c1 = slice(G * H, (G + 1) * H)                      # W1 cols
        cb0 = G * H + (1 if G == 3 else 0)
        c2 = slice(cb0, cb0 + H)                            # W2B cols
        for base in (0, 64):
            W1[base:base + 50, c1] = Whh1[blk, :].T * gm
            W1[base + 50, c1] = Wih1[blk, 0] * gm
            W1[base + 51, c1] = b1[blk] * gm
            W2A[base:base + 50, c2] = Wih2[blk, :].T * gm
            W2B[base:base + 50, c2] = Whh2[blk, :].T * gm
            W2B[base + 50, c2] = b2[blk] * gm
    for base in (0, 64):
        W2B[base:base + 50, 150] = Wl[0, :]
        W2B[base + 50, 150] = bl[0]
        WLY[base:base + 50, 0] = Wl[0, :]
        WLY[base + 50, 0] = bl[0]
    return (W1.astype(np.float16), W2A.astype(np.float16),
            W2B.astype(np.float16), WLY.astype(np.float16))


_NC_CACHE = {}
_last_in_maps = None


def _run(x, Wih1, Whh1, bih1, bhh1, Wih2, Whh2, bih2, bhh2, Wl, bl, future,
         trace=False):
    x = np.asarray(x, np.float32)
    nB, T = x.shape
    FUT = int(future)
    assert nB == B, (nB, B)

    key = (T, FUT)
    if key not in _NC_CACHE:
        _NC_CACHE[key] = _build_nc(T, FUT)
    nc = _NC_CACHE[key]

    W1, W2A, W2B, WLY = _prep_weights(
        np.asarray(Wih1, np.float32), np.asarray(Whh1, np.float32),
        np.asarray(bih1, np.float32), np.asarray(bhh1, np.float32),
        np.asarray(Wih2, np.float32), np.asarray(Whh2, np.float32),
        np.asarray(bih2, np.float32), np.asarray(bhh2, np.float32),
        np.asarray(Wl, np.float32), np.asarray(bl, np.float32))

    in_maps = []
    for c in range(NCORES):
        xc = np.ascontiguousarray(x[c * BC:(c + 1) * BC, :].T).astype(np.float16)
        in_maps.append({"xT": xc, "W1": W1, "W2A": W2A, "W2B": W2B,
                        "WLY": WLY, "ONES": np.ones((1, HALF), np.float16)})

    global _last_in_maps
    _last_in_maps = in_maps
    res = run_bass_kernel_spmd(nc, in_maps, list(range(NCORES)), trace=trace)
    out = np.empty((B, T + FUT), np.float32)
    for c in range(NCORES):
        out[c * BC:(c + 1) * BC, :] = res.results[c]["yT"].T.astype(np.float32)
    return out, res


def kernel(**inputs):
    out, _ = _run(**inputs)
    return out

